# revision 15
# baseline (speedup 1.0000x reference)
"""MD-LSTM (4-direction 2D LSTM) Trainium2 Bass kernel, v2.

Sharding (8 NeuronCores, SPMD): core c handles direction (c % 4) with batch
half (c // 4); the 16-batch half is split into TWO sub-scans of 8 (A, B) so
one sub-scan's pointwise tail overlaps the other's matmuls.

Per sub-scan the H,W recurrence runs as anti-diagonal wavefronts: 159 steps,
<=32 diagonal cells x 8 batch = <=256 matmul columns per step, contracting
[x(+ones); h_up; h_lf] against [w0(+bias); u0; u1] (bf16) into PSUM, then the
LSTM cell update with fp32 c-state.

v2 changes vs v1:
- x input (with a ones row for the bias) lives entirely in SBUF in a compact
  diagonal layout (one upfront DMA per sub-scan) -> no per-step x DMAs, no
  Sync-queue head-blocking, x-projection matmuls never data-starve.
- bias folded into the x matmul via the ones row -> per-gate ACT bias gone.
- gate order [l, f, i, o, g~] with g~ = 2g (host scales g columns x2):
  tanh(g) = 2*sigmoid(2g) - 1, so ALL five gates use sigmoid ->
  two fused ACTIVATEs per sub-scan-step ([l,f] then [i,o,g~]) instead of five.
- i*g computed as t = i*sig(g~); ig = 2t - i (one tensor_tensor + one fused
  scalar_tensor_tensor on DVE).
- whole c-chain on DVE (GpSimd only does the off-critical-path c_up - c_lf).
- PSUM: per-gate 1KB regions; sub-scan A double-buffered (4+6 = 5 banks),
  B single-buffered (3 banks) = exactly 8 banks; kills matmul WAR stalls on
  ACT reads for A, and B's x-matmuls are emitted right before B's u-matmuls
  so their WAR wait is subsumed by the h dependency.

Self-contained: hardcodes all shapes; reads no files.
"""
import numpy as np

import concourse.bass as bass
import concourse.bacc as bacc
import concourse.mybir as mybir
import concourse.tile as tile
from concourse import bass_utils

B, CIN, H, W, OC = 32, 64, 32, 128, 128
CINB = CIN + 2              # +two ones rows (bias hi+lo split)
NSTEP = H + W - 1           # 159
BQ = 8                      # batch per sub-scan
GQ = 16                     # state guard cols (16 -> 32B-aligned h windows)
SWQ = H * BQ                # 256 max window cols
HWQ = GQ + SWQ              # guard + slots = 272
FP = mybir.dt.float32
BF = mybir.dt.bfloat16

# host-side gate reorder to [l, f, i, o, g] (reference order [i, f, g, o, l])
GATE_ORDER = [4, 1, 0, 3, 2]
J_L, J_F, J_I, J_O, J_G = 0, 1, 2, 3, 4


def _window(d):
    return max(0, d - (W - 1)), min(d, H - 1)


def _offsets():
    off = [0]
    for d in range(NSTEP):
        y0, y1 = _window(d)
        off.append(off[-1] + (y1 - y0 + 1) * BQ)
    return off


OFF = _offsets()
TOTC = OFF[-1]              # 32768 compact cols per sub-scan


# schedule-shape knobs (sim-tuned)
# g_mode: "tanh"  -> separate tanh ACT for the g gate (baseline numerics)
#         "sig2"  -> g~=2g, tanh(g)=2*sig(g~)-1 (fewer ACTs; bf16 sigma
#                    quantization costs ~0.3% extra error)
CFG = {
    "g_mode": "tanh",
    "dummies": 10,   # dummy LDWEIGHTS per period to hold the PE p-state
}


def build_kernel(cfg=None):
    cfg = dict(CFG, **(cfg or {}))
    nc = bacc.Bacc("TRN2", target_bir_lowering=False, debug=False, num_devices=8)

    xs_d = [nc.dram_tensor(f"x_diag{s}", [CINB, TOTC], BF, kind="ExternalInput")
            for s in range(2)]
    w0_d = nc.dram_tensor("w0", [CINB, 5 * OC], FP, kind="ExternalInput")
    u0_d = nc.dram_tensor("u0", [OC, 5 * OC], FP, kind="ExternalInput")
    u1_d = nc.dram_tensor("u1", [OC, 5 * OC], FP, kind="ExternalInput")
    outs_d = [nc.dram_tensor(f"out_diag{s}", [OC, TOTC], BF,
                             kind="ExternalOutput") for s in range(2)]

    act = mybir.ActivationFunctionType
    alu = mybir.AluOpType

    with tile.TileContext(nc) as tc:
        with (
            tc.tile_pool(name="const", bufs=1) as cpool,
            tc.tile_pool(name="psum", bufs=1, space="PSUM") as ppool,
        ):
            # ---- weights (fp32 load -> bf16 cast once) ----
            w0s = cpool.tile([CINB, 5 * OC], FP, tag="w0")
            u0s = cpool.tile([OC, 5 * OC], FP, tag="u0")
            u1s = cpool.tile([OC, 5 * OC], FP, tag="u1")
            nc.sync.dma_start(w0s[:, :], w0_d.ap())
            nc.sync.dma_start(u0s[:, :], u0_d.ap())
            nc.sync.dma_start(u1s[:, :], u1_d.ap())
            w0m = cpool.tile([CINB, 5 * OC], BF, tag="w0b")
            u0m = cpool.tile([OC, 5 * OC], BF, tag="u0b")
            u1m = cpool.tile([OC, 5 * OC], BF, tag="u1b")
            nc.vector.tensor_copy(w0m[:, :], w0s[:, :])
            nc.vector.tensor_copy(u0m[:, :], u0s[:, :])
            nc.vector.tensor_copy(u1m[:, :], u1s[:, :])

            # ---- x input resident in SBUF (compact diagonal layout) ----
            xs = [cpool.tile([CINB, TOTC], BF, tag=f"xs{s}", name=f"xs{s}")
                  for s in range(2)]
            for s in range(2):
                nc.sync.dma_start(xs[s][:, :], xs_d[s].ap())

            # ---- per-sub-scan double-buffered state ----
            hb = [[cpool.tile([OC, HWQ], BF, tag=f"hb{s}{k}", name=f"hb{s}{k}")
                   for k in range(2)] for s in range(2)]
            cb = [[cpool.tile([OC, HWQ], FP, tag=f"cb{s}{k}", name=f"cb{s}{k}")
                   for k in range(2)] for s in range(2)]
            for s in range(2):
                for k in range(2):
                    nc.vector.memset(hb[s][k][:, :], 0.0)
                    nc.vector.memset(cb[s][k][:, :], 0.0)

            # sigmoid outputs [l,f,i,o,g~], double-buffered per sub-scan
            g5 = [[cpool.tile([OC, 5, SWQ], BF, tag=f"g5{s}{k}", name=f"g5{s}{k}")
                   for k in range(2)] for s in range(2)]
            # c_up - c_lf, double-buffered per sub-scan
            dcx = [[cpool.tile([OC, SWQ], FP, tag=f"dcx{s}{k}", name=f"dcx{s}{k}")
                    for k in range(2)] for s in range(2)]
            # single-buffered scratch (same-queue ordering makes WAR free)
            mt = [cpool.tile([OC, SWQ], FP, tag=f"mt{s}", name=f"mt{s}")
                  for s in range(2)]
            tt = [cpool.tile([OC, SWQ], BF, tag=f"tt{s}", name=f"tt{s}")
                  for s in range(2)]
            ig = [cpool.tile([OC, SWQ], BF, tag=f"ig{s}", name=f"ig{s}")
                  for s in range(2)]
            th = [cpool.tile([OC, SWQ], BF, tag=f"th{s}", name=f"th{s}")
                  for s in range(2)]

            # ---- PSUM: 4 banks per sub-scan (8 exactly) ----
            # psLF: l and f EACH own a bank ([128, 2, 512] fp32, gate j at
            # [:, j, 0:nwin]) so both x-matmuls prefetch (concurrent groups
            # in distinct banks). psIOG: i,o share a bank (groups strictly
            # sequential: x-o is emitted after u1-i closes i's group), g~
            # owns the padded second bank (x-g~ prefetchable).
            psLF = [ppool.tile([OC, 2, 2 * SWQ], FP, tag=f"psLF{s}",
                               name=f"psLF{s}") for s in range(2)]
            psIOG = [ppool.tile([OC, 3, SWQ], FP, tag=f"psIOG{s}",
                                name=f"psIOG{s}") for s in range(2)]

            def ps_gate(s, j, nwin):
                if j < 2:
                    return psLF[s][:, j, 0:nwin]
                return psIOG[s][:, j - 2, 0:nwin]

            def mm_x(s, e, gates):
                """x-projection (+bias) for step e, start=True."""
                y0, y1 = _window(e)
                nwin = (y1 - y0 + 1) * BQ
                rhs = xs[s][:, OFF[e]:OFF[e] + nwin]
                for j in gates:
                    nc.tensor.matmul(ps_gate(s, j, nwin),
                                     w0m[:, j * OC:(j + 1) * OC], rhs,
                                     start=True, stop=False)

            def mm_u(s, d, prev, gates):
                y0, y1 = _window(d)
                nwin = (y1 - y0 + 1) * BQ
                lo = GQ + y0 * BQ
                rhs_up = hb[s][prev][:, lo - BQ: lo - BQ + nwin]
                rhs_lf = hb[s][prev][:, lo: lo + nwin]
                for j in gates:   # l,f first: frees the sig-lf ACT early
                    nc.tensor.matmul(ps_gate(s, j, nwin),
                                     u0m[:, j * OC:(j + 1) * OC], rhs_up,
                                     start=False, stop=False)
                    nc.tensor.matmul(ps_gate(s, j, nwin),
                                     u1m[:, j * OC:(j + 1) * OC], rhs_lf,
                                     start=False, stop=True)

            def sig_lf(s, d):
                y0, y1 = _window(d)
                nwin = (y1 - y0 + 1) * BQ
                nc.scalar.activation(g5[s][d % 2][:, 0:2, 0:nwin],
                                     psLF[s][:, :, 0:nwin], act.Sigmoid)

            def sig_iog(s, d):
                y0, y1 = _window(d)
                nwin = (y1 - y0 + 1) * BQ
                if cfg["g_mode"] == "sig2":
                    nc.scalar.activation(g5[s][d % 2][:, 2:5, 0:nwin],
                                         psIOG[s][:, :, 0:nwin], act.Sigmoid)
                else:
                    nc.scalar.activation(g5[s][d % 2][:, 2:4, 0:nwin],
                                         psIOG[s][:, 0:2, 0:nwin], act.Sigmoid)
                    nc.scalar.activation(g5[s][d % 2][:, 4, 0:nwin],
                                         psIOG[s][:, 2, 0:nwin], act.Tanh)

            def dve_m(s, d, cur, prev):
                """m = f * (l * dcx + c_lf)"""
                y0, y1 = _window(d)
                nwin = (y1 - y0 + 1) * BQ
                lo = GQ + y0 * BQ
                g = g5[s][d % 2]
                m = mt[s][:, 0:nwin]
                c_lf = cb[s][prev][:, lo: lo + nwin]
                nc.vector.tensor_tensor(m, g[:, J_L, 0:nwin],
                                        dcx[s][d % 2][:, 0:nwin], alu.mult)
                nc.vector.tensor_tensor(m, m, c_lf, alu.add)
                nc.vector.tensor_tensor(m, g[:, J_F, 0:nwin], m, alu.mult)

            def dve_tigc(s, d, cur, prev):
                """ig = i*g; c = m + ig"""
                y0, y1 = _window(d)
                nwin = (y1 - y0 + 1) * BQ
                lo = GQ + y0 * BQ
                g = g5[s][d % 2]
                i_ = g[:, J_I, 0:nwin]
                ig_ = ig[s][:, 0:nwin]
                if cfg["g_mode"] == "sig2":
                    t_ = tt[s][:, 0:nwin]
                    nc.vector.tensor_tensor(t_, i_, g[:, J_G, 0:nwin], alu.mult)
                    nc.vector.scalar_tensor_tensor(ig_, t_, 2.0, i_,
                                                   alu.mult, alu.subtract)
                else:
                    nc.vector.tensor_tensor(ig_, i_, g[:, J_G, 0:nwin], alu.mult)
                cw = cb[s][cur][:, lo: lo + nwin]
                nc.vector.tensor_tensor(cw, mt[s][:, 0:nwin], ig_, alu.add)

            def tanh_c(s, d, cur):
                y0, y1 = _window(d)
                nwin = (y1 - y0 + 1) * BQ
                lo = GQ + y0 * BQ
                nc.scalar.activation(th[s][:, 0:nwin],
                                     cb[s][cur][:, lo: lo + nwin], act.Tanh)

            def h_out(s, d, cur):
                y0, y1 = _window(d)
                nwin = (y1 - y0 + 1) * BQ
                lo = GQ + y0 * BQ
                hw = hb[s][cur][:, lo: lo + nwin]
                nc.vector.tensor_tensor(hw, g5[s][d % 2][:, J_O, 0:nwin],
                                        th[s][:, 0:nwin], alu.mult)
                nc.sync.dma_start(outs_d[s].ap()[:, OFF[d]:OFF[d] + nwin], hw)

            def dcx_next(s, e, cur):
                """c_up - c_lf for step e from state written at step e-1."""
                y0, y1 = _window(e)
                nwin = (y1 - y0 + 1) * BQ
                lo = GQ + y0 * BQ
                c_up = cb[s][cur][:, lo - BQ: lo - BQ + nwin]
                c_lf = cb[s][cur][:, lo: lo + nwin]
                nc.gpsimd.tensor_tensor(dcx[s][e % 2][:, 0:nwin],
                                        c_up, c_lf, alu.subtract)

            # ---- prologue: step-0 x-proj (l,f,i,g~) and dcx (zero-state) ----
            for s in range(2):
                dcx_next(s, 0, 1)   # reads cb[s][1] (zeros)
                mm_x(s, 0, (J_L, J_F, J_I, J_G))

            # ---- wavefront loop ----
            def u_phase(s, d, prev):
                mm_u(s, d, prev, (J_L, J_F))
                sig_lf(s, d)
                mm_u(s, d, prev, (J_I,))
                mm_x(s, d, (J_O,))      # i's group closed; o reuses its bank
                mm_u(s, d, prev, (J_O, J_G))
                sig_iog(s, d)

            for d in range(NSTEP):
                cur, prev = d % 2, (d + 1) % 2
                u_phase(0, d, prev)
                dve_m(0, d, cur, prev)
                u_phase(1, d, prev)
                dve_tigc(0, d, cur, prev)
                dve_m(1, d, cur, prev)
                tanh_c(0, d, cur)
                h_out(0, d, cur)
                dve_tigc(1, d, cur, prev)
                tanh_c(1, d, cur)
                h_out(1, d, cur)
                if d + 1 < NSTEP:
                    dcx_next(0, d + 1, cur)
                    dcx_next(1, d + 1, cur)
                    mm_x(0, d + 1, (J_L, J_F, J_I, J_G))
                    mm_x(1, d + 1, (J_L, J_F, J_I, J_G))
                    # dummy weight loads: dependency-anchored to this
                    # period's sig-lf output so they fill the PE queue
                    # during the h(d) wait and keep the p-state up
                    for _ in range(cfg["dummies"]):
                        nc.tensor.ldweights(g5[0][d % 2][:, 0, 0:OC])

    nc.compile()
    return nc


_NC_CACHE = {}


def _get_nc():
    if "nc" not in _NC_CACHE:
        _NC_CACHE["nc"] = build_kernel()
    return _NC_CACHE["nc"]


def _flip(x, d):
    if d == 1:
        return x[:, :, :, ::-1]
    if d == 2:
        return x[:, :, ::-1, :]
    if d == 3:
        return x[:, :, ::-1, ::-1]
    return x


def _gate_scale():
    """Column scale for [w0;b]/u0/u1: g~ = 2g in sig2 mode."""
    gsc = np.ones((5 * OC,), np.float32)
    if CFG["g_mode"] == "sig2":
        gsc[J_G * OC:(J_G + 1) * OC] = 2.0
    return gsc


def _col_index():
    """COLS[y, x, b] = compact column of cell (y, x) batch-lane b."""
    cols = np.empty((H, W, BQ), np.int64)
    for y in range(H):
        for x in range(W):
            d = x + y
            y0, _ = _window(d)
            base = OFF[d] + (y - y0) * BQ
            cols[y, x] = base + np.arange(BQ)
    return cols


_COLS = _col_index()


def _make_x_diag(x_nat):
    """(CINB, H, W, BQ) -> (CINB, TOTC) compact diagonal layout."""
    arr = np.empty((CINB, TOTC), np.float32)
    arr[:, _COLS.reshape(-1)] = x_nat.reshape(CINB, -1)
    return arr


def _decode(out_diag):
    """(OC, TOTC) fp32 -> (BQ, OC, H, W)"""
    return out_diag[:, _COLS].transpose(3, 0, 1, 2)


def kernel(x, w0, u0, u1, b, trace=False, _res=[None]):
    import ml_dtypes
    x = np.asarray(x, np.float32)
    w0 = np.asarray(w0, np.float32)
    u0 = np.asarray(u0, np.float32)
    u1 = np.asarray(u1, np.float32)
    b = np.asarray(b, np.float32)

    perm = np.concatenate([np.arange(g * OC, (g + 1) * OC) for g in GATE_ORDER])
    gsc = _gate_scale()

    in_maps = []
    for c in range(8):
        dirn, half = c % 4, c // 4
        xsf = _flip(x[half * 16:(half + 1) * 16], dirn)          # (16,CIN,H,W)
        x_nat = np.empty((CINB, H, W, 16), np.float32)
        x_nat[:CIN] = xsf.transpose(1, 2, 3, 0)
        x_nat[CIN:] = 1.0
        # bias split into bf16 hi + residual lo rows so the bf16 weight
        # quantization of the bias is second-order
        bp = b[dirn][perm] * gsc
        b_hi = bp.astype(ml_dtypes.bfloat16).astype(np.float32)
        w0p = np.empty((CINB, 5 * OC), np.float32)
        w0p[:CIN] = w0[dirn][:, perm] * gsc
        w0p[CIN] = b_hi
        w0p[CIN + 1] = bp - b_hi
        m = {
            "w0": np.ascontiguousarray(w0p),
            "u0": np.ascontiguousarray(u0[dirn][:, perm] * gsc),
            "u1": np.ascontiguousarray(u1[dirn][:, perm] * gsc),
        }
        for s in range(2):
            m[f"x_diag{s}"] = _make_x_diag(
                x_nat[:, :, :, s * BQ:(s + 1) * BQ]).astype(ml_dtypes.bfloat16)
        in_maps.append(m)

    nc = _get_nc()
    res = bass_utils.run_bass_kernel_spmd(nc, in_maps, list(range(8)), trace=trace)
    _res[0] = res

    out = np.empty((B, 4, OC, H, W), np.float32)
    for c in range(8):
        dirn, half = c % 4, c // 4
        for s in range(2):
            od = np.asarray(res.results[c][f"out_diag{s}"]).astype(np.float32)
            lo = half * 16 + s * BQ
            out[lo:lo + BQ, dirn] = _decode(od)
    return out


# revision 18
# speedup vs baseline: 1.0741x; 1.0741x over previous
"""MD-LSTM (4-direction 2D LSTM) Trainium2 Bass kernel, v2.

Sharding (8 NeuronCores, SPMD): core c handles direction (c % 4) with batch
half (c // 4); the 16-batch half is split into TWO sub-scans of 8 (A, B) so
one sub-scan's pointwise tail overlaps the other's matmuls.

Per sub-scan the H,W recurrence runs as anti-diagonal wavefronts: 159 steps,
<=32 diagonal cells x 8 batch = <=256 matmul columns per step, contracting
[x(+ones); h_up; h_lf] against [w0(+bias); u0; u1] (bf16) into PSUM, then the
LSTM cell update with fp32 c-state.

v2 changes vs v1:
- x input (with a ones row for the bias) lives entirely in SBUF in a compact
  diagonal layout (one upfront DMA per sub-scan) -> no per-step x DMAs, no
  Sync-queue head-blocking, x-projection matmuls never data-starve.
- bias folded into the x matmul via the ones row -> per-gate ACT bias gone.
- gate order [l, f, i, o, g~] with g~ = 2g (host scales g columns x2):
  tanh(g) = 2*sigmoid(2g) - 1, so ALL five gates use sigmoid ->
  two fused ACTIVATEs per sub-scan-step ([l,f] then [i,o,g~]) instead of five.
- i*g computed as t = i*sig(g~); ig = 2t - i (one tensor_tensor + one fused
  scalar_tensor_tensor on DVE).
- whole c-chain on DVE (GpSimd only does the off-critical-path c_up - c_lf).
- PSUM: per-gate 1KB regions; sub-scan A double-buffered (4+6 = 5 banks),
  B single-buffered (3 banks) = exactly 8 banks; kills matmul WAR stalls on
  ACT reads for A, and B's x-matmuls are emitted right before B's u-matmuls
  so their WAR wait is subsumed by the h dependency.

Self-contained: hardcodes all shapes; reads no files.
"""
import numpy as np

import concourse.bass as bass
import concourse.bacc as bacc
import concourse.mybir as mybir
import concourse.tile as tile
from concourse import bass_utils

B, CIN, H, W, OC = 32, 64, 32, 128, 128
CINB = CIN + 2              # +two ones rows (bias hi+lo split)
NSTEP = H + W - 1           # 159
BQ = 8                      # batch per sub-scan
GQ = 16                     # state guard cols (16 -> 32B-aligned h windows)
SWQ = H * BQ                # 256 max window cols
HWQ = GQ + SWQ              # guard + slots = 272
FP = mybir.dt.float32
BF = mybir.dt.bfloat16
F16 = mybir.dt.float16

# host-side gate reorder to [l, f, i, g, o] (reference order [i, f, g, o, l])
GATE_ORDER = [4, 1, 0, 2, 3]
J_L, J_F, J_I, J_G, J_O = 0, 1, 2, 3, 4


def _window(d):
    return max(0, d - (W - 1)), min(d, H - 1)


def _offsets():
    off = [0]
    for d in range(NSTEP):
        y0, y1 = _window(d)
        off.append(off[-1] + (y1 - y0 + 1) * BQ)
    return off


OFF = _offsets()
TOTC = OFF[-1]              # 32768 compact cols per sub-scan


# schedule-shape knobs (sim-tuned)
# g_mode: "tanh"  -> separate tanh ACT for the g gate (baseline numerics)
#         "sig2"  -> g~=2g, tanh(g)=2*sig(g~)-1 (fewer ACTs; bf16 sigma
#                    quantization costs ~0.3% extra error)
CFG = {
    "g_mode": "sig2",
    "dummies": 0,    # dummy LDWEIGHTS per period to hold the PE p-state
}


def build_kernel(cfg=None):
    cfg = dict(CFG, **(cfg or {}))
    nc = bacc.Bacc("TRN2", target_bir_lowering=False, debug=False, num_devices=8)

    xs_d = [nc.dram_tensor(f"x_diag{s}", [CINB, TOTC], BF, kind="ExternalInput")
            for s in range(2)]
    w0_d = nc.dram_tensor("w0", [CINB, 5 * OC], FP, kind="ExternalInput")
    u0_d = nc.dram_tensor("u0", [OC, 5 * OC], FP, kind="ExternalInput")
    u1_d = nc.dram_tensor("u1", [OC, 5 * OC], FP, kind="ExternalInput")
    outs_d = [nc.dram_tensor(f"out_diag{s}", [OC, TOTC], BF,
                             kind="ExternalOutput") for s in range(2)]

    act = mybir.ActivationFunctionType
    alu = mybir.AluOpType

    with tile.TileContext(nc) as tc:
        with (
            tc.tile_pool(name="const", bufs=1) as cpool,
            tc.tile_pool(name="psum", bufs=1, space="PSUM") as ppool,
        ):
            # ---- weights (fp32 load -> bf16 cast once) ----
            w0s = cpool.tile([CINB, 5 * OC], FP, tag="w0")
            u0s = cpool.tile([OC, 5 * OC], FP, tag="u0")
            u1s = cpool.tile([OC, 5 * OC], FP, tag="u1")
            nc.sync.dma_start(w0s[:, :], w0_d.ap())
            nc.sync.dma_start(u0s[:, :], u0_d.ap())
            nc.sync.dma_start(u1s[:, :], u1_d.ap())
            w0m = cpool.tile([CINB, 5 * OC], BF, tag="w0b")
            u0m = cpool.tile([OC, 5 * OC], BF, tag="u0b")
            u1m = cpool.tile([OC, 5 * OC], BF, tag="u1b")
            nc.vector.tensor_copy(w0m[:, :], w0s[:, :])
            nc.vector.tensor_copy(u0m[:, :], u0s[:, :])
            nc.vector.tensor_copy(u1m[:, :], u1s[:, :])

            # ---- x input resident in SBUF (compact diagonal layout) ----
            xs = [cpool.tile([CINB, TOTC], BF, tag=f"xs{s}", name=f"xs{s}")
                  for s in range(2)]
            for s in range(2):
                nc.sync.dma_start(xs[s][:, :], xs_d[s].ap())

            # ---- per-sub-scan double-buffered state ----
            hb = [[cpool.tile([OC, HWQ], BF, tag=f"hb{s}{k}", name=f"hb{s}{k}")
                   for k in range(2)] for s in range(2)]
            cb = [[cpool.tile([OC, HWQ], FP, tag=f"cb{s}{k}", name=f"cb{s}{k}")
                   for k in range(2)] for s in range(2)]
            for s in range(2):
                for k in range(2):
                    nc.vector.memset(hb[s][k][:, :], 0.0)
                    nc.vector.memset(cb[s][k][:, :], 0.0)

            # gate nonlinearity outputs, double-buffered per sub-scan:
            # g2 = [l, f] bf16; g2h = [i, sig(g~)] fp16 (fp16's finer
            # quantization near 0.5 keeps the 2*sig-1 reconstruction of
            # tanh(g) accurate); go = o bf16
            g2 = [[cpool.tile([OC, 2, SWQ], BF, tag=f"g2{s}{k}", name=f"g2{s}{k}")
                   for k in range(2)] for s in range(2)]
            g2h = [[cpool.tile([OC, 2, SWQ], F16, tag=f"g2h{s}{k}",
                    name=f"g2h{s}{k}") for k in range(2)] for s in range(2)]
            go = [[cpool.tile([OC, SWQ], BF, tag=f"go{s}{k}", name=f"go{s}{k}")
                   for k in range(2)] for s in range(2)]
            # c_up - c_lf, double-buffered per sub-scan
            dcx = [[cpool.tile([OC, SWQ], FP, tag=f"dcx{s}{k}", name=f"dcx{s}{k}")
                    for k in range(2)] for s in range(2)]
            # single-buffered scratch (same-queue ordering makes WAR free)
            mt = [cpool.tile([OC, SWQ], FP, tag=f"mt{s}", name=f"mt{s}")
                  for s in range(2)]
            tt = [cpool.tile([OC, SWQ], F16, tag=f"tt{s}", name=f"tt{s}")
                  for s in range(2)]
            ig = [cpool.tile([OC, SWQ], F16, tag=f"ig{s}", name=f"ig{s}")
                  for s in range(2)]
            th = [cpool.tile([OC, SWQ], BF, tag=f"th{s}", name=f"th{s}")
                  for s in range(2)]

            # ---- PSUM: 4 banks per sub-scan (8 exactly) ----
            # psLF [128,2,512]: l owns bank 0; f and o SHARE bank 1 (f at
            # cols 0:256, o at 256:512; their groups are strictly
            # sequential: x-o is emitted after u1-f closes f's group).
            # psIG [128,2,512]: i and g~ each own a bank. So x-matmuls for
            # l, f, i, g~ all prefetch one step ahead as PE filler, and the
            # paired sigmoids read [l,f] / [i,g~] as strided 3D APs.
            psLF = [ppool.tile([OC, 2, 2 * SWQ], FP, tag=f"psLF{s}",
                               name=f"psLF{s}") for s in range(2)]
            psIG = [ppool.tile([OC, 2, 2 * SWQ], FP, tag=f"psIG{s}",
                               name=f"psIG{s}") for s in range(2)]

            def ps_gate(s, j, nwin):
                if j == J_L:
                    return psLF[s][:, 0, 0:nwin]
                if j == J_F:
                    return psLF[s][:, 1, 0:nwin]
                if j == J_O:
                    return psLF[s][:, 1, SWQ:SWQ + nwin]
                if j == J_I:
                    return psIG[s][:, 0, 0:nwin]
                return psIG[s][:, 1, 0:nwin]

            def mm_x(s, e, gates):
                """x-projection (+bias) for step e, start=True."""
                y0, y1 = _window(e)
                nwin = (y1 - y0 + 1) * BQ
                rhs = xs[s][:, OFF[e]:OFF[e] + nwin]
                for j in gates:
                    nc.tensor.matmul(ps_gate(s, j, nwin),
                                     w0m[:, j * OC:(j + 1) * OC], rhs,
                                     start=True, stop=False)

            def mm_u(s, d, prev, gates):
                y0, y1 = _window(d)
                nwin = (y1 - y0 + 1) * BQ
                lo = GQ + y0 * BQ
                rhs_up = hb[s][prev][:, lo - BQ: lo - BQ + nwin]
                rhs_lf = hb[s][prev][:, lo: lo + nwin]
                for j in gates:   # l,f first: frees the sig-lf ACT early
                    nc.tensor.matmul(ps_gate(s, j, nwin),
                                     u0m[:, j * OC:(j + 1) * OC], rhs_up,
                                     start=False, stop=False)
                    nc.tensor.matmul(ps_gate(s, j, nwin),
                                     u1m[:, j * OC:(j + 1) * OC], rhs_lf,
                                     start=False, stop=True)

            def sig_lf(s, d):
                y0, y1 = _window(d)
                nwin = (y1 - y0 + 1) * BQ
                nc.scalar.activation(g2[s][d % 2][:, :, 0:nwin],
                                     psLF[s][:, :, 0:nwin], act.Sigmoid)

            def sig_ig(s, d):
                y0, y1 = _window(d)
                nwin = (y1 - y0 + 1) * BQ
                nc.scalar.activation(g2h[s][d % 2][:, :, 0:nwin],
                                     psIG[s][:, :, 0:nwin], act.Sigmoid)

            def sig_o(s, d):
                y0, y1 = _window(d)
                nwin = (y1 - y0 + 1) * BQ
                nc.scalar.activation(go[s][d % 2][:, 0:nwin],
                                     psLF[s][:, 1, SWQ:SWQ + nwin], act.Sigmoid)

            def dve_m(s, d, cur, prev):
                """m = f * (l * dcx + c_lf)"""
                y0, y1 = _window(d)
                nwin = (y1 - y0 + 1) * BQ
                lo = GQ + y0 * BQ
                g = g2[s][d % 2]
                m = mt[s][:, 0:nwin]
                c_lf = cb[s][prev][:, lo: lo + nwin]
                nc.vector.tensor_tensor(m, g[:, 0, 0:nwin],
                                        dcx[s][d % 2][:, 0:nwin], alu.mult)
                nc.vector.tensor_tensor(m, m, c_lf, alu.add)
                nc.vector.tensor_tensor(m, g[:, 1, 0:nwin], m, alu.mult)

            def dve_tigc(s, d, cur, prev):
                """ig = i*g; c = m + ig"""
                y0, y1 = _window(d)
                nwin = (y1 - y0 + 1) * BQ
                lo = GQ + y0 * BQ
                g = g2h[s][d % 2]
                i_ = g[:, 0, 0:nwin]
                ig_ = ig[s][:, 0:nwin]
                t_ = tt[s][:, 0:nwin]
                nc.vector.tensor_tensor(t_, i_, g[:, 1, 0:nwin], alu.mult)
                nc.vector.scalar_tensor_tensor(ig_, t_, 2.0, i_,
                                               alu.mult, alu.subtract)
                cw = cb[s][cur][:, lo: lo + nwin]
                nc.vector.tensor_tensor(cw, mt[s][:, 0:nwin], ig_, alu.add)

            def tanh_c(s, d, cur):
                y0, y1 = _window(d)
                nwin = (y1 - y0 + 1) * BQ
                lo = GQ + y0 * BQ
                nc.scalar.activation(th[s][:, 0:nwin],
                                     cb[s][cur][:, lo: lo + nwin], act.Tanh)

            def h_out(s, d, cur):
                y0, y1 = _window(d)
                nwin = (y1 - y0 + 1) * BQ
                lo = GQ + y0 * BQ
                hw = hb[s][cur][:, lo: lo + nwin]
                nc.vector.tensor_tensor(hw, go[s][d % 2][:, 0:nwin],
                                        th[s][:, 0:nwin], alu.mult)
                nc.sync.dma_start(outs_d[s].ap()[:, OFF[d]:OFF[d] + nwin], hw)

            def dcx_next(s, e, cur):
                """c_up - c_lf for step e from state written at step e-1."""
                y0, y1 = _window(e)
                nwin = (y1 - y0 + 1) * BQ
                lo = GQ + y0 * BQ
                c_up = cb[s][cur][:, lo - BQ: lo - BQ + nwin]
                c_lf = cb[s][cur][:, lo: lo + nwin]
                nc.gpsimd.tensor_tensor(dcx[s][e % 2][:, 0:nwin],
                                        c_up, c_lf, alu.subtract)

            # ---- prologue: step-0 x-proj (l,f,i,g~) and dcx (zero-state) ----
            for s in range(2):
                dcx_next(s, 0, 1)   # reads cb[s][1] (zeros)
                mm_x(s, 0, (J_L, J_F, J_I, J_G))

            # ---- wavefront loop ----
            def u_phase(s, d, prev):
                mm_u(s, d, prev, (J_L, J_F))
                sig_lf(s, d)
                mm_x(s, d, (J_O,))      # f's group closed; o reuses its bank
                mm_u(s, d, prev, (J_I, J_G))
                sig_ig(s, d)
                mm_u(s, d, prev, (J_O,))
                sig_o(s, d)

            for d in range(NSTEP):
                cur, prev = d % 2, (d + 1) % 2
                u_phase(0, d, prev)
                dve_m(0, d, cur, prev)
                u_phase(1, d, prev)
                dve_tigc(0, d, cur, prev)
                dve_m(1, d, cur, prev)
                tanh_c(0, d, cur)
                h_out(0, d, cur)
                dve_tigc(1, d, cur, prev)
                tanh_c(1, d, cur)
                h_out(1, d, cur)
                if d + 1 < NSTEP:
                    dcx_next(0, d + 1, cur)
                    dcx_next(1, d + 1, cur)
                    mm_x(0, d + 1, (J_L, J_F, J_I, J_G))
                    mm_x(1, d + 1, (J_L, J_F, J_I, J_G))
                    # dummy weight loads: dependency-anchored to this
                    # period's sig-lf output so they fill the PE queue
                    # during the h(d) wait and keep the p-state up
                    for _ in range(cfg["dummies"]):
                        nc.tensor.ldweights(g5[0][d % 2][:, 0, 0:OC])

    nc.compile()
    return nc


_NC_CACHE = {}


def _get_nc():
    if "nc" not in _NC_CACHE:
        _NC_CACHE["nc"] = build_kernel()
    return _NC_CACHE["nc"]


def _flip(x, d):
    if d == 1:
        return x[:, :, :, ::-1]
    if d == 2:
        return x[:, :, ::-1, :]
    if d == 3:
        return x[:, :, ::-1, ::-1]
    return x


def _gate_scale():
    """Column scale for [w0;b]/u0/u1: g~ = 2g in sig2 mode."""
    gsc = np.ones((5 * OC,), np.float32)
    if CFG["g_mode"] == "sig2":
        gsc[J_G * OC:(J_G + 1) * OC] = 2.0
    return gsc


def _col_index():
    """COLS[y, x, b] = compact column of cell (y, x) batch-lane b."""
    cols = np.empty((H, W, BQ), np.int64)
    for y in range(H):
        for x in range(W):
            d = x + y
            y0, _ = _window(d)
            base = OFF[d] + (y - y0) * BQ
            cols[y, x] = base + np.arange(BQ)
    return cols


_COLS = _col_index()


def _make_x_diag(x_nat):
    """(CINB, H, W, BQ) -> (CINB, TOTC) compact diagonal layout."""
    arr = np.empty((CINB, TOTC), np.float32)
    arr[:, _COLS.reshape(-1)] = x_nat.reshape(CINB, -1)
    return arr


def _decode(out_diag):
    """(OC, TOTC) fp32 -> (BQ, OC, H, W)"""
    return out_diag[:, _COLS].transpose(3, 0, 1, 2)


def kernel(x, w0, u0, u1, b, trace=False, _res=[None]):
    import ml_dtypes
    x = np.asarray(x, np.float32)
    w0 = np.asarray(w0, np.float32)
    u0 = np.asarray(u0, np.float32)
    u1 = np.asarray(u1, np.float32)
    b = np.asarray(b, np.float32)

    perm = np.concatenate([np.arange(g * OC, (g + 1) * OC) for g in GATE_ORDER])
    gsc = _gate_scale()

    in_maps = []
    for c in range(8):
        dirn, half = c % 4, c // 4
        xsf = _flip(x[half * 16:(half + 1) * 16], dirn)          # (16,CIN,H,W)
        x_nat = np.empty((CINB, H, W, 16), np.float32)
        x_nat[:CIN] = xsf.transpose(1, 2, 3, 0)
        x_nat[CIN:] = 1.0
        # bias split into bf16 hi + residual lo rows so the bf16 weight
        # quantization of the bias is second-order
        bp = b[dirn][perm] * gsc
        b_hi = bp.astype(ml_dtypes.bfloat16).astype(np.float32)
        w0p = np.empty((CINB, 5 * OC), np.float32)
        w0p[:CIN] = w0[dirn][:, perm] * gsc
        w0p[CIN] = b_hi
        w0p[CIN + 1] = bp - b_hi
        m = {
            "w0": np.ascontiguousarray(w0p),
            "u0": np.ascontiguousarray(u0[dirn][:, perm] * gsc),
            "u1": np.ascontiguousarray(u1[dirn][:, perm] * gsc),
        }
        for s in range(2):
            m[f"x_diag{s}"] = _make_x_diag(
                x_nat[:, :, :, s * BQ:(s + 1) * BQ]).astype(ml_dtypes.bfloat16)
        in_maps.append(m)

    nc = _get_nc()
    res = bass_utils.run_bass_kernel_spmd(nc, in_maps, list(range(8)), trace=trace)
    _res[0] = res

    out = np.empty((B, 4, OC, H, W), np.float32)
    for c in range(8):
        dirn, half = c % 4, c // 4
        for s in range(2):
            od = np.asarray(res.results[c][f"out_diag{s}"]).astype(np.float32)
            lo = half * 16 + s * BQ
            out[lo:lo + BQ, dirn] = _decode(od)
    return out
